# revision 6
# baseline (speedup 1.0000x reference)
"""Trainium2 Bass kernel for causal self-attention (B=2, S=2048, D=1024, H=16).

Sharding: 8 cores = 2 (batch) x 4 (head groups of 4 heads) — data parallel on
batch, tensor parallel on heads. Each core computes, for its batch b and its
4 heads (256 of the 1024 model dims):

  qT/kT = Wq_slice^T x^T            transposed layouts [head_dim, seq], fp16
  v     = x Wv_slice                natural layout [seq, head_dim], fp16
  per head pair (2 heads share the 128 partitions):
    scoresT[kv, q] blocks on PE (two row-packed K=64 matmuls),
    exp on ACT (psum -> fp16 sbuf), causal mask multiply on DVE (fp16 2x),
    P^T V + replicated ones-row denominators on PE (col-packed M=64),
    normalize: reciprocal_approx_fast + one tensor_mul.
  oT_partial = Wo_slice^T attnT     [1024, seq] fp32 partial

Host: feeds x^T and fp16 weight slices, sums the 4 partials per batch
(the "all-reduce" of the o-projection), transposes, adds bo.

All matmuls run in fp16 (1 cyc/row on PE) with fp32 PSUM accumulation;
softmax scale 1/sqrt(64) is folded into Wq on the host. Projections are
interleaved with the (ACT-bound) attention loop in program order, input DMAs
are split per contraction tile so compute starts early, and diagonal blocks
are column-sliced to skip fully-masked work.
"""

import numpy as np

import concourse.bacc as bacc
import concourse.tile as tile
from concourse import mybir
from concourse.bass_utils import run_bass_kernel_spmd

B, S, D, H = 2, 2048, 1024, 16
HD = D // H          # 64
P = 128
NCORES = 8
GROUPS = 4           # head groups (tensor parallel)
HPG = H // GROUPS    # 4 heads per group
CD = HPG * HD        # 256 local head dims per core
QT = 512             # q tile (matmul free dim)
KT = 128             # kv tile (psum partition dim)
NQT = S // QT        # 4
NKT = S // KT        # 16
KD = D // P          # 8 contraction tiles over the model dim

F32 = mybir.dt.float32
F16 = mybir.dt.float16
EXP = mybir.ActivationFunctionType.Exp

_NC_CACHE = {}


def _build_nc():
    if "nc" in _NC_CACHE:
        return _NC_CACHE["nc"]
    nc = bacc.Bacc()
    xt = nc.declare_dram_parameter("xt", [D, S], F16, isOutput=False)
    wq = nc.declare_dram_parameter("wq", [D, CD], F16, isOutput=False)
    wk = nc.declare_dram_parameter("wk", [D, CD], F16, isOutput=False)
    wv = nc.declare_dram_parameter("wv", [D, CD], F16, isOutput=False)
    wo = nc.declare_dram_parameter("wo", [CD, D], F16, isOutput=False)
    bq = nc.declare_dram_parameter("bq", [CD], F32, isOutput=False)
    bk = nc.declare_dram_parameter("bk", [CD], F32, isOutput=False)
    bv = nc.declare_dram_parameter("bv", [HPG, HD], F32, isOutput=False)
    msk = nc.declare_dram_parameter("msk", [4, P, 2 * QT], F16, isOutput=False)
    ot = nc.declare_dram_parameter("ot", [D, S], F32, isOutput=True)

    import concourse.bass as bass

    with tile.TileContext(nc) as tc:
        with tc.tile_pool(name="consts", bufs=1) as consts, \
             tc.tile_pool(name="work", bufs=3) as work, \
             tc.tile_pool(name="ps_s", bufs=2, space="PSUM") as ps_s, \
             tc.tile_pool(name="ps_av", bufs=1, space="PSUM") as ps_av, \
             tc.tile_pool(name="ps_po", bufs=2, space="PSUM") as ps_po:

            # ---- constant / persistent SBUF tensors ----
            xt_sb = consts.tile([P, KD, S], F16)
            wq_sb = consts.tile([P, KD, CD], F16)
            wk_sb = consts.tile([P, KD, CD], F16)
            wv_sb = consts.tile([P, KD, CD], F16)
            wo_sb = consts.tile([P, 2, D], F16)
            bq_sb = consts.tile([P, 2], F32)
            bk_sb = consts.tile([P, 2], F32)
            bv_sb = consts.tile([P, HPG, HD], F32)
            ones_sb = consts.tile([P, 64], F16)
            msk_sb = consts.tile([P, 4, 2 * QT], F16)
            qT_sb = consts.tile([P, 2, S], F16)
            kT_sb = consts.tile([P, 2, S], F16)
            v_sb = consts.tile([P, NKT, HPG, HD], F16)
            aT_sb = consts.tile([P, 2, NQT, QT], F16)

            # ---- input DMAs, split per contraction tile so compute can
            # start as soon as the first chunks land ----
            nc.sync.dma_start(out=bq_sb, in_=bq[:].rearrange("(m p) -> p m", p=P))
            nc.sync.dma_start(out=bk_sb, in_=bk[:].rearrange("(m p) -> p m", p=P))
            bv_ap = bv[:, :]
            bv_bc = bass.AP(tensor=bv_ap.tensor, offset=bv_ap.offset,
                            ap=[[0, P]] + list(bv_ap.ap))
            nc.gpsimd.dma_start(out=bv_sb, in_=bv_bc)
            nc.vector.memset(ones_sb, 1.0)
            wq_r = wq[:, :].rearrange("(k p) c -> p k c", p=P)
            wk_r = wk[:, :].rearrange("(k p) c -> p k c", p=P)
            wv_r = wv[:, :].rearrange("(k p) c -> p k c", p=P)
            xt_r = xt[:, :].rearrange("(k p) s -> p k s", p=P)
            for kt in range(KD):
                nc.sync.dma_start(out=wq_sb[:, kt, :], in_=wq_r[:, kt, :])
                nc.sync.dma_start(out=wk_sb[:, kt, :], in_=wk_r[:, kt, :])
                nc.sync.dma_start(out=wv_sb[:, kt, :], in_=wv_r[:, kt, :])
                nc.sync.dma_start(out=xt_sb[:, kt, :], in_=xt_r[:, kt, :])
                if kt == 0:
                    nc.sync.dma_start(
                        out=msk_sb, in_=msk[:, :, :].rearrange("r p c -> p r c"))
            nc.sync.dma_start(out=wo_sb,
                              in_=wo[:, :].rearrange("(g p) e -> p g e", p=P))

            # ---- helpers ----
            def proj_qk(w_sb, b_sb, dst, mt):
                for half in range(2):
                    ps_pair = [ps_po.tile([P, QT], F32, tag="po",
                                          name=f"pp_{half}_{nt}")
                               for nt in range(2)]
                    for kt in range(KD):
                        lhs = w_sb[:, kt, mt * P:(mt + 1) * P]
                        for j in range(2):
                            nt = 2 * half + j
                            nc.tensor.matmul(
                                ps_pair[j], lhs,
                                xt_sb[:, kt, nt * QT:(nt + 1) * QT],
                                start=(kt == 0), stop=(kt == KD - 1))
                    for j in range(2):
                        nt = 2 * half + j
                        nc.vector.tensor_scalar_add(
                            dst[:, mt, nt * QT:(nt + 1) * QT], ps_pair[j],
                            b_sb[:, mt:mt + 1])

            def proj_v(jt0, jt1):
                for jt in range(jt0, jt1):
                    ps = ps_po.tile([P, QT], F32, tag="po", name="ps_v")
                    for kt in range(KD):
                        nc.tensor.matmul(
                            ps[:, :CD], xt_sb[:, kt, jt * P:(jt + 1) * P],
                            wv_sb[:, kt, :],
                            start=(kt == 0), stop=(kt == KD - 1))
                    nc.vector.tensor_add(
                        v_sb[:, jt, :, :],
                        ps[:, :CD].rearrange("p (h d) -> p h d", h=HPG), bv_sb)

            def attention(t, g):
                n_kv = 4 * (t + 1)
                av = ps_av.tile([P, QT], F32, tag="av", name="av")
                den = ps_av.tile([P, QT], F32, tag="den", name="den")
                for kv in range(n_kv):
                    r = kv - 4 * t
                    v0 = KT * r if r >= 1 else 0    # first valid q col
                    s = ps_s.tile([P, 2 * QT], F32, tag="s", name="s")
                    for idx in range(2):
                        p0 = 64 * idx
                        nc.tensor.matmul(
                            s[:, idx * QT + v0:(idx + 1) * QT],
                            kT_sb[p0:p0 + 64, g, kv * KT:(kv + 1) * KT],
                            qT_sb[p0:p0 + 64, g, t * QT + v0:(t + 1) * QT],
                            start=True, stop=True)
                    p_t = work.tile([P, 2 * QT], F16, tag="pt", name="p_t")
                    if r < 1:
                        nc.scalar.activation(p_t, s, EXP)
                    else:
                        for idx in range(2):
                            sl = slice(idx * QT + v0, (idx + 1) * QT)
                            nc.scalar.activation(p_t[:, sl], s[:, sl], EXP)
                    if r >= 0:
                        for idx in range(2):
                            sl = slice(idx * QT + v0, (idx + 1) * QT)
                            nc.vector.tensor_mul(p_t[:, sl], p_t[:, sl],
                                                 msk_sb[:, r, sl])
                    for idx in range(2):
                        h = 2 * g + idx
                        rhs = p_t[:, idx * QT + v0:(idx + 1) * QT]
                        nc.tensor.matmul(
                            av[64 * idx:64 * idx + 64, v0:],
                            v_sb[:, kv, h, :], rhs,
                            start=(kv == 0), stop=(kv == n_kv - 1),
                            skip_group_check=True,
                            tile_position=(0, 64 * idx))
                        nc.tensor.matmul(
                            den[64 * idx:64 * idx + 64, v0:],
                            ones_sb, rhs,
                            start=(kv == 0), stop=(kv == n_kv - 1),
                            skip_group_check=True,
                            tile_position=(0, 64 * idx))
                # normalize: aT[:, g, t, :] = av * (1 / den)
                rc = work.tile([P, QT], F32, tag="rc", name="rc")
                nc.vector.reciprocal_approx_fast(rc, den)
                nc.vector.tensor_mul(aT_sb[:, g, t, :], av, rc)

            def oproj(t):
                for mt_e in range(D // P):
                    ps = ps_po.tile([P, QT], F32, tag="po", name="ps_o")
                    for g in range(2):
                        nc.tensor.matmul(
                            ps, wo_sb[:, g, mt_e * P:(mt_e + 1) * P],
                            aT_sb[:, g, t, :],
                            start=(g == 0), stop=(g == 1))
                    ot_t = work.tile([P, QT], F32, tag="ot", name="ot_t")
                    nc.vector.tensor_copy(ot_t, ps)
                    nc.sync.dma_start(
                        out=ot[mt_e * P:(mt_e + 1) * P, t * QT:(t + 1) * QT],
                        in_=ot_t)

            # ---- interleaved schedule: attention (ACT-bound) absorbs the
            # projections and o-proj in the PE slack ----
            proj_qk(wq_sb, bq_sb, qT_sb, 0)
            proj_qk(wk_sb, bk_sb, kT_sb, 0)
            proj_v(0, 4)
            attention(0, 0)
            proj_qk(wq_sb, bq_sb, qT_sb, 1)
            proj_qk(wk_sb, bk_sb, kT_sb, 1)
            proj_v(4, 8)
            attention(0, 1)
            oproj(0)
            proj_v(8, 12)
            attention(1, 0)
            attention(1, 1)
            oproj(1)
            proj_v(12, 16)
            attention(2, 0)
            attention(2, 1)
            oproj(2)
            attention(3, 0)
            attention(3, 1)
            oproj(3)

    nc.compile()
    _NC_CACHE["nc"] = nc
    return nc


def _make_masks():
    # msk[r, p, c] for the 4 diagonal kv offsets r: valid iff p <= (c % 512) - 128 r
    m = np.zeros((4, P, 2 * QT), dtype=np.float16)
    pp = np.arange(P)[:, None]
    cc = np.arange(QT)[None, :]
    for r in range(4):
        half = (pp <= cc - KT * r).astype(np.float16)
        m[r, :, :QT] = half
        m[r, :, QT:] = half
    return m


def _in_maps(x, Wq, bq, Wk, bk, Wv, bv, Wo):
    scale = np.float32(1.0 / np.sqrt(HD))
    masks = _make_masks()
    maps = []
    for core in range(NCORES):
        b, g = divmod(core, GROUPS)
        csl = slice(g * CD, (g + 1) * CD)
        maps.append({
            "xt": np.ascontiguousarray(x[b].T).astype(np.float16),
            "wq": np.ascontiguousarray(Wq[:, csl] * scale).astype(np.float16),
            "wk": np.ascontiguousarray(Wk[:, csl]).astype(np.float16),
            "wv": np.ascontiguousarray(Wv[:, csl]).astype(np.float16),
            "wo": np.ascontiguousarray(Wo[csl, :]).astype(np.float16),
            "bq": np.ascontiguousarray(bq[csl] * scale).astype(np.float32),
            "bk": np.ascontiguousarray(bk[csl]).astype(np.float32),
            "bv": np.ascontiguousarray(bv[csl]).reshape(HPG, HD).astype(np.float32),
            "msk": masks,
        })
    return maps


def kernel_with_results(x, Wq, bq, Wk, bk, Wv, bv, Wo, bo, trace=False):
    nc = _build_nc()
    maps = _in_maps(x, Wq, bq, Wk, bk, Wv, bv, Wo)
    kwargs = {}
    if trace:
        kwargs = dict(trace=True, trace_cores=[0])
    res = run_bass_kernel_spmd(nc, maps, core_ids=list(range(NCORES)), **kwargs)
    out = np.zeros((B, S, D), dtype=np.float32)
    for b in range(B):
        acc = np.zeros((D, S), dtype=np.float32)
        for g in range(GROUPS):
            acc += res.results[b * GROUPS + g]["ot"]
        out[b] = acc.T + np.asarray(bo, dtype=np.float32)[None, :]
    return out, res


def kernel(x, Wq, bq, Wk, bk, Wv, bv, Wo, bo):
    out, _ = kernel_with_results(x, Wq, bq, Wk, bk, Wv, bv, Wo, bo, trace=False)
    return out
